# revision 5
# baseline (speedup 1.0000x reference)
"""Viterbi decode (BiLSTM-CRF) Bass kernel for Trainium2.

Strategy: the forward recursion (T=8192 serial steps over a [2048, 2048]
max-plus matvec) runs in a For_i loop on one NeuronCore using a fused
custom DVE op (add + max-reduce in one pass, 16 x [128, 2048] tiles per
step). The new forward-variable row is rebuilt via transpose + a
partition-broadcast DMA through DRAM (exact f32 — no PE arithmetic on the
values). History rows land in internal DRAM; a second on-device pass
computes all backpointers for the final best tag (argmax via the DVE
max/max_index instructions). Host only assembles the final path.
"""
import numpy as np

import concourse.bacc as bacc
import concourse.bass as bass
import concourse.mybir as mybir
import concourse.tile as tile
from concourse.bass_utils import run_bass_kernel_spmd

# ---- runtime registration of the fused max-plus DVE op ----
import concourse.dve_ops as dve_ops
from concourse.dve_ops import DveOp, OPS, CUSTOM_DVE_SPECS, _SUB_OPCODE_FOR_NAME
from concourse.dve_spec import Spec, Src0, Src1, C0, maxx, lower, _has_src1
from concourse.dve_uop import DveOpSpec


def _register(name, spec):
    if name in _SUB_OPCODE_FOR_NAME:
        return next(op for op in OPS if op.name == name)
    shas = {}
    for ver in ("v3", "v4"):
        try:
            uops = lower(spec, ver=ver)
            shas[ver] = DveOpSpec(
                name=name, opcode=0, uops=uops, rd1_en=_has_src1(spec)
            ).sha(ver)
        except Exception:
            pass
    op = DveOp(name, spec, subdim=False, uops_sha=shas)
    OPS.append(op)
    CUSTOM_DVE_SPECS[name] = spec
    _SUB_OPCODE_FOR_NAME[name] = max(_SUB_OPCODE_FOR_NAME.values()) + 1
    return op


def _ref_vitmax(in0, in1, s0, s1, imm2):
    b = (in0.astype(np.float32) + np.asarray(in1, np.float32)).astype(np.float32)
    P = b.shape[0]
    acc = np.maximum(
        np.broadcast_to(np.asarray(s0, np.float32).reshape(-1, 1), (P, 1)),
        b.reshape(P, -1).max(axis=-1, keepdims=True),
    ).astype(np.float32)
    return b, acc


VITMAX = _register(
    "VITMAX",
    Spec(body=Src0 + Src1, accum=maxx, accum_init=C0, reference=_ref_vitmax),
)

T, K, NT = 8192, 2048, 16
NBP = T // 128  # backptr tiles
FMIN = float(np.finfo(np.float32).min)
f32 = mybir.dt.float32
u32 = mybir.dt.uint32

_CACHE = {}


def _bcast_src(ap_row, nparts=128):
    """[1, K] DRAM row -> [nparts, K] partition-broadcast source AP."""
    return bass.AP(tensor=ap_row.tensor, offset=ap_row.offset,
                   ap=[[0, nparts]] + list(ap_row.ap[1:]))


def _bcast_fill(nc, dst, src_row):
    """Fill dst [128, K] with src_row [1, K] via 4 parallel partition-chunk DMAs."""
    for q in range(4):
        nc.sync.dma_start(out=dst[32 * q:32 * (q + 1), :],
                          in_=_bcast_src(src_row, 32))


def _build(end_tag):
    nc = bacc.Bacc("TRN2", target_bir_lowering=False, debug=False, num_devices=1)
    A_d = nc.dram_tensor("transition", [K, K], f32, kind="ExternalInput")
    F_d = nc.dram_tensor("features", [T, K], f32, kind="ExternalInput")
    I_d = nc.dram_tensor("init_fv", [1, K], f32, kind="ExternalInput")
    E_d = nc.dram_tensor("eye", [128, 128], f32, kind="ExternalInput")
    BP_d = nc.dram_tensor("bp", [NBP, 128], u32, kind="ExternalOutput")
    EV_d = nc.dram_tensor("ev8", [1, 8], f32, kind="ExternalOutput")
    IX_d = nc.dram_tensor("ix8", [1, 8], u32, kind="ExternalOutput")
    H_d = nc.dram_tensor("hist", [T, K], f32)
    FV_d = nc.dram_tensor("fvcur", [1, K], f32)
    AB_d = nc.dram_tensor("abest", [1, K], f32)

    with tile.TileContext(nc) as tc:
        with (tc.tile_pool(name="sbuf", bufs=1) as pool,
              tc.tile_pool(name="psum", bufs=1, space="PSUM") as psum):
            a_tiles = []
            for m in range(NT):
                t_ = pool.tile([128, K], f32, name=f"A{m}", tag=f"A{m}")
                a_tiles.append(t_)
                nc.sync.dma_start(out=t_[:], in_=A_d[128 * m:128 * (m + 1), :])
            G = pool.tile([128, NT], f32, name="G")
            junk = pool.tile([128, K], f32, name="junk")
            fvb = pool.tile([128, K], f32, name="fvb")
            feat16 = pool.tile([16, 128], f32, name="feat16")
            T_fv = pool.tile([16, 128], f32, name="T_fv")
            eye = pool.tile([128, 128], f32, name="eye")
            T_ps = psum.tile([16, 128], f32, name="T_ps")
            nc.sync.dma_start(out=eye[:], in_=E_d[:, :])
            _bcast_fill(nc, fvb, I_d[0:1, :])

            # ---- forward recursion ----
            with tc.For_i(0, T, 1) as i:
                nc.sync.dma_start(
                    out=feat16[:],
                    in_=F_d[bass.ds(i, 1), :].rearrange("o (p n) -> (o p) n", p=16))
                for m in range(NT):
                    nc.vector._custom_dve(
                        VITMAX, out=junk[:], in0=a_tiles[m][:], in1=fvb[:],
                        s0=FMIN, accum_out=G[:, m:m + 1])
                nc.tensor.transpose(T_ps[:], G[:], eye[:])
                nc.vector.scalar_tensor_tensor(
                    out=T_fv[:], in0=T_ps[:], scalar=0.0, in1=feat16[:],
                    op0=mybir.AluOpType.add, op1=mybir.AluOpType.add)
                nc.sync.dma_start(out=FV_d[0:1, :], in_=T_fv[:])
                nc.sync.dma_start(out=H_d[bass.ds(i, 1), :], in_=T_fv[:])
                _bcast_fill(nc, fvb, FV_d[0:1, :])

            # ---- final tag: end_var = fv_T + A[end_tag, :] ----
            arow = pool.tile([1, K], f32, name="arow")
            ev = pool.tile([1, K], f32, name="ev")
            ev8 = pool.tile([1, 8], f32, name="ev8")
            ix8 = pool.tile([1, 8], u32, name="ix8")
            lastfv = pool.tile([1, K], f32, name="lastfv")
            nc.sync.dma_start(out=arow[:], in_=A_d[end_tag:end_tag + 1, :])
            nc.sync.dma_start(out=lastfv[:], in_=H_d[T - 1:T, :])
            nc.vector.scalar_tensor_tensor(
                out=ev[:], in0=lastfv[:], scalar=0.0, in1=arow[:],
                op0=mybir.AluOpType.add, op1=mybir.AluOpType.add)
            nc.vector.max(ev8[:], ev[:])
            nc.vector.max_index(ix8[:], ev8[:], ev[:])
            nc.sync.dma_start(out=EV_d[0:1, :], in_=ev8[:])
            nc.sync.dma_start(out=IX_d[0:1, :], in_=ix8[:])

            # ---- gather A[best, :] and broadcast it ----
            abest = pool.tile([1, K], f32, name="abest")
            abestb = pool.tile([128, K], f32, name="abestb")
            reg = nc.sync.value_load(ix8[0:1, 0:1])
            nc.sync.dma_start(out=abest[:], in_=A_d[bass.ds(reg, 1), :])
            nc.sync.dma_start(out=AB_d[0:1, :], in_=abest[:])
            _bcast_fill(nc, abestb, AB_d[0:1, :])

            # ---- backpointers: bp[t] = argmax_j(hist[t] + A[best, j]) ----
            htile = pool.tile([128, K], f32, name="htile")
            sums = pool.tile([128, K], f32, name="sums")
            m8 = pool.tile([128, 8], f32, name="m8")
            i8 = pool.tile([128, 8], u32, name="i8")
            with tc.For_i(0, NBP, 1) as j:
                nc.sync.dma_start(out=htile[:], in_=H_d[bass.ts(j, 128), :])
                nc.vector.scalar_tensor_tensor(
                    out=sums[:], in0=htile[:], scalar=0.0, in1=abestb[:],
                    op0=mybir.AluOpType.add, op1=mybir.AluOpType.add)
                nc.vector.max(m8[:], sums[:])
                nc.vector.max_index(i8[:], m8[:], sums[:])
                nc.sync.dma_start(out=BP_d[bass.ds(j, 1), :], in_=i8[:, 0:1])
    nc.compile()
    return nc


def kernel(features, transition, start_tag, end_tag):
    features = np.ascontiguousarray(features, dtype=np.float32)
    transition = np.ascontiguousarray(transition, dtype=np.float32)
    st = int(np.asarray(start_tag))
    et = int(np.asarray(end_tag))
    assert features.shape == (T, K) and transition.shape == (K, K)

    if et not in _CACHE:
        _CACHE[et] = _build(et)
    nc = _CACHE[et]

    init = np.full((1, K), -10000.0, np.float32)
    init[0, st] = 0.0
    inmap = {
        "transition": transition,
        "features": features,
        "init_fv": init,
        "eye": np.eye(128, dtype=np.float32),
    }
    res = run_bass_kernel_spmd(nc, [inmap], [0])
    out = res.results[0]
    best = int(out["ix8"][0, 0])
    best_score = np.float32(out["ev8"][0, 0])
    bp = out["bp"].reshape(-1).astype(np.int32)  # bp[t] for hist row t (= fv_{t+1})
    path = np.concatenate([bp[0:T - 1], np.array([best], np.int32)])
    return path, best_score


# revision 7
# speedup vs baseline: 1.0456x; 1.0456x over previous
"""Viterbi decode (BiLSTM-CRF) Bass kernel for Trainium2.

Strategy: the forward recursion (T=8192 serial steps over a [2048, 2048]
max-plus matvec) runs in a For_i loop on one NeuronCore using a fused
custom DVE op (add + max-reduce in one pass, 16 x [128, 2048] tiles per
step). The new forward-variable row is rebuilt via transpose + a
partition-broadcast DMA through DRAM (exact f32 — no PE arithmetic on the
values). History rows land in internal DRAM; a second on-device pass
computes all backpointers for the final best tag (argmax via the DVE
max/max_index instructions). Host only assembles the final path.
"""
import numpy as np

import concourse.bacc as bacc
import concourse.bass as bass
import concourse.mybir as mybir
import concourse.tile as tile
from concourse.bass_utils import run_bass_kernel_spmd

# ---- runtime registration of the fused max-plus DVE op ----
import concourse.dve_ops as dve_ops
from concourse.dve_ops import DveOp, OPS, CUSTOM_DVE_SPECS, _SUB_OPCODE_FOR_NAME
from concourse.dve_spec import Spec, Src0, Src1, C0, maxx, lower, _has_src1
from concourse.dve_uop import DveOpSpec


def _register(name, spec):
    if name in _SUB_OPCODE_FOR_NAME:
        return next(op for op in OPS if op.name == name)
    shas = {}
    for ver in ("v3", "v4"):
        try:
            uops = lower(spec, ver=ver)
            shas[ver] = DveOpSpec(
                name=name, opcode=0, uops=uops, rd1_en=_has_src1(spec)
            ).sha(ver)
        except Exception:
            pass
    op = DveOp(name, spec, subdim=False, uops_sha=shas)
    OPS.append(op)
    CUSTOM_DVE_SPECS[name] = spec
    _SUB_OPCODE_FOR_NAME[name] = max(_SUB_OPCODE_FOR_NAME.values()) + 1
    return op


def _ref_vitmax(in0, in1, s0, s1, imm2):
    b = (in0.astype(np.float32) + np.asarray(in1, np.float32)).astype(np.float32)
    P = b.shape[0]
    acc = np.maximum(
        np.broadcast_to(np.asarray(s0, np.float32).reshape(-1, 1), (P, 1)),
        b.reshape(P, -1).max(axis=-1, keepdims=True),
    ).astype(np.float32)
    return b, acc


VITMAX = _register(
    "VITMAX",
    Spec(body=Src0 + Src1, accum=maxx, accum_init=C0, reference=_ref_vitmax),
)

T, K, NT = 8192, 2048, 16
NBP = T // 128  # backptr tiles
FMIN = float(np.finfo(np.float32).min)
f32 = mybir.dt.float32
u32 = mybir.dt.uint32

_CACHE = {}


def _bcast_src(ap_row, nparts=128):
    """[1, K] DRAM row -> [nparts, K] partition-broadcast source AP."""
    return bass.AP(tensor=ap_row.tensor, offset=ap_row.offset,
                   ap=[[0, nparts]] + list(ap_row.ap[1:]))


def _bcast_fill(nc, dst, src_row):
    """Fill dst [128, K] with src_row [1, K] via 4 parallel partition-chunk DMAs."""
    for q in range(4):
        nc.sync.dma_start(out=dst[32 * q:32 * (q + 1), :],
                          in_=_bcast_src(src_row, 32))


def _build(end_tag):
    nc = bacc.Bacc("TRN2", target_bir_lowering=False, debug=False, num_devices=1)
    A_d = nc.dram_tensor("transition", [K, K], f32, kind="ExternalInput")
    F_d = nc.dram_tensor("features", [T, K], f32, kind="ExternalInput")
    I_d = nc.dram_tensor("init_fv", [1, K], f32, kind="ExternalInput")
    E_d = nc.dram_tensor("eye", [128, 128], f32, kind="ExternalInput")
    BP_d = nc.dram_tensor("bp", [NBP, 128], u32, kind="ExternalOutput")
    EV_d = nc.dram_tensor("ev8", [1, 8], f32, kind="ExternalOutput")
    IX_d = nc.dram_tensor("ix8", [1, 8], u32, kind="ExternalOutput")
    H_d = nc.dram_tensor("hist", [T, K], f32)
    FV_d = nc.dram_tensor("fvcur", [1, K], f32)
    AB_d = nc.dram_tensor("abest", [1, K], f32)

    with tile.TileContext(nc) as tc:
        with (tc.tile_pool(name="sbuf", bufs=1) as pool,
              tc.tile_pool(name="psum", bufs=1, space="PSUM") as psum):
            a_tiles = []
            for m in range(NT):
                t_ = pool.tile([128, K], f32, name=f"A{m}", tag=f"A{m}")
                a_tiles.append(t_)
                nc.sync.dma_start(out=t_[:], in_=A_d[128 * m:128 * (m + 1), :])
            G = pool.tile([128, NT], f32, name="G")
            junk = pool.tile([128, K], f32, name="junk")
            fvb = pool.tile([128, K], f32, name="fvb")
            eye = pool.tile([128, 128], f32, name="eye")
            nc.sync.dma_start(out=eye[:], in_=E_d[:, :])
            fvb2 = pool.tile([128, K], f32, name="fvb2")
            Th = [pool.tile([8, 128], f32, name=f"Th{q}") for q in range(4)]
            Fh = [pool.tile([8, 128], f32, name=f"Fh{q}") for q in range(4)]
            Tp = [psum.tile([8, 128], f32, name=f"Tp{q}") for q in range(4)]
            FVh = [nc.dram_tensor(f"fvh{q}", [1, K // 2], f32) for q in range(4)]
            _bcast_fill(nc, fvb, I_d[0:1, :])

            # ---- forward recursion: 2 steps per iteration, fvb double-buffered,
            # each half's transpose/broadcast overlaps the other half's DVE ops ----
            def _substep(src, dst, q0, row):
                for h in range(2):
                    q = q0 + h
                    cols = slice(1024 * h, 1024 * (h + 1))
                    for m in range(8 * h, 8 * h + 8):
                        nc.vector._custom_dve(
                            VITMAX, out=junk[:], in0=a_tiles[m][:], in1=src[:],
                            s0=FMIN, accum_out=G[:, m:m + 1])
                    nc.tensor.transpose(Tp[q][:], G[:, 8 * h:8 * h + 8], eye[:])
                    nc.vector.scalar_tensor_tensor(
                        out=Th[q][:], in0=Tp[q][:], scalar=0.0, in1=Fh[q][:],
                        op0=mybir.AluOpType.add, op1=mybir.AluOpType.add)
                    nc.sync.dma_start(out=FVh[q][0:1, :], in_=Th[q][:])
                    nc.sync.dma_start(out=H_d[row, cols], in_=Th[q][:])
                    for p in range(2):
                        nc.sync.dma_start(
                            out=dst[64 * p:64 * (p + 1), cols],
                            in_=_bcast_src(FVh[q][0:1, :], 64))

            with tc.For_i(0, T // 2, 1) as i:
                for q in range(4):
                    nc.sync.dma_start(
                        out=Fh[q][:],
                        in_=F_d[bass.ds(i * 2 + q // 2, 1), 1024 * (q % 2):1024 * (q % 2 + 1)]
                        .rearrange("o (p n) -> (o p) n", p=8))
                _substep(fvb, fvb2, 0, bass.ds(i * 2, 1))
                _substep(fvb2, fvb, 2, bass.ds(i * 2 + 1, 1))

            # ---- final tag: end_var = fv_T + A[end_tag, :] ----
            arow = pool.tile([1, K], f32, name="arow")
            ev = pool.tile([1, K], f32, name="ev")
            ev8 = pool.tile([1, 8], f32, name="ev8")
            ix8 = pool.tile([1, 8], u32, name="ix8")
            lastfv = pool.tile([1, K], f32, name="lastfv")
            nc.sync.dma_start(out=arow[:], in_=A_d[end_tag:end_tag + 1, :])
            nc.sync.dma_start(out=lastfv[:], in_=H_d[T - 1:T, :])
            nc.vector.scalar_tensor_tensor(
                out=ev[:], in0=lastfv[:], scalar=0.0, in1=arow[:],
                op0=mybir.AluOpType.add, op1=mybir.AluOpType.add)
            nc.vector.max(ev8[:], ev[:])
            nc.vector.max_index(ix8[:], ev8[:], ev[:])
            nc.sync.dma_start(out=EV_d[0:1, :], in_=ev8[:])
            nc.sync.dma_start(out=IX_d[0:1, :], in_=ix8[:])

            # ---- gather A[best, :] and broadcast it ----
            abest = pool.tile([1, K], f32, name="abest")
            abestb = pool.tile([128, K], f32, name="abestb")
            reg = nc.sync.value_load(ix8[0:1, 0:1])
            nc.sync.dma_start(out=abest[:], in_=A_d[bass.ds(reg, 1), :])
            nc.sync.dma_start(out=AB_d[0:1, :], in_=abest[:])
            _bcast_fill(nc, abestb, AB_d[0:1, :])

            # ---- backpointers: bp[t] = argmax_j(hist[t] + A[best, j]) ----
            htile = pool.tile([128, K], f32, name="htile")
            sums = junk
            m8 = pool.tile([128, 8], f32, name="m8")
            i8 = pool.tile([128, 8], u32, name="i8")
            with tc.For_i(0, NBP, 1) as j:
                nc.sync.dma_start(out=htile[:], in_=H_d[bass.ts(j, 128), :])
                nc.vector.scalar_tensor_tensor(
                    out=sums[:], in0=htile[:], scalar=0.0, in1=abestb[:],
                    op0=mybir.AluOpType.add, op1=mybir.AluOpType.add)
                nc.vector.max(m8[:], sums[:])
                nc.vector.max_index(i8[:], m8[:], sums[:])
                nc.sync.dma_start(out=BP_d[bass.ds(j, 1), :], in_=i8[:, 0:1])
    nc.compile()
    return nc


def kernel(features, transition, start_tag, end_tag):
    features = np.ascontiguousarray(features, dtype=np.float32)
    transition = np.ascontiguousarray(transition, dtype=np.float32)
    st = int(np.asarray(start_tag))
    et = int(np.asarray(end_tag))
    assert features.shape == (T, K) and transition.shape == (K, K)

    if et not in _CACHE:
        _CACHE[et] = _build(et)
    nc = _CACHE[et]

    init = np.full((1, K), -10000.0, np.float32)
    init[0, st] = 0.0
    inmap = {
        "transition": transition,
        "features": features,
        "init_fv": init,
        "eye": np.eye(128, dtype=np.float32),
    }
    res = run_bass_kernel_spmd(nc, [inmap], [0])
    out = res.results[0]
    best = int(out["ix8"][0, 0])
    best_score = np.float32(out["ev8"][0, 0])
    bp = out["bp"].reshape(-1).astype(np.int32)  # bp[t] for hist row t (= fv_{t+1})
    path = np.concatenate([bp[0:T - 1], np.array([best], np.int32)])
    return path, best_score
